# revision 23
# baseline (speedup 1.0000x reference)
"""Trainium2 Bass kernel for nn_BTT2_76733885710855.

Computes: W[a,b,c,d] = sum_r L[a,b,c,r] * R[r,b,c,d]  (reshaped [4096, 4096])
          out = x @ W.T + bias                          x: [8, 2048, 4096]

Strategy: data-parallel over the batch dim (8 cores, x[k] per core).
Per core:
  Phase A: assemble Wt = W.T  ([i=(c,d), o=(a,b)]) into DRAM scratch via
           k=8 TensorE matmuls packed 4x2 with tile_position
           (per (b,c): Wt block[d, a] = Rbc[r,d].T @ LbcT[r,a]).
  Phase B: out = X @ Wt + bias, tiled 128x512x128, with X transposed
           on-chip through the PE (identity transpose).
L is host-permuted to LT[r,b,c,a] (pure relayout of a replicated weight)
so its DMA loads are contiguous.
"""

import os
import numpy as np
from contextlib import ExitStack

import concourse.bass as bass
import concourse.mybir as mybir
import concourse.tile as tile
from concourse import bacc
from concourse.bass_utils import run_bass_kernel_spmd
from concourse.masks import make_identity

F32 = mybir.dt.float32
F32R = mybir.dt.float32r

N_CORES = 8
S = 2048          # rows per core
KD = 4096         # in dim  (i = c*64 + d)
OD = 4096         # out dim (o = a*64 + b)
S_CHUNK = 1024    # rows of Xt resident at once

# matmul input dtype for the big GEMM ("f32" exact / "f32r" fast)
MM_DTYPE = os.environ.get("BTT2_MM_DTYPE", "f32r")


def _mm_dt():
    return F32R if MM_DTYPE == "f32r" else F32


def _emit_assembly(ctx, tc, LT, R, wt, wdt):
    """Phase A: build wt[4096, 4096] = W.T in DRAM.

    wt[c*64+d, a*64+b] = sum_r R[r,b,c,d] * LT[r,b,c,a]
    """
    del wdt  # packed variant predates the f32r plumbing (currently unused)
    nc = tc.nc
    qpool = ctx.enter_context(tc.tile_pool(name="quads", bufs=1))
    slabs = ctx.enter_context(tc.tile_pool(name="slab", bufs=2))
    apsum = ctx.enter_context(tc.tile_pool(name="apsum", bufs=3, space="PSUM"))

    for co in range(8):  # c-octet: c in [8*co, 8*co+8)
        # Load 16 R-quads + 16 LT-quads, each [128, 8(c), 64].
        # Quad tile q holds b in {4q..4q+3}; b-local bq sits at partition
        # strip 32*bq, rows [32*bq, 32*bq+8) = r. Rows 8..31 of each strip
        # are never read (k=8 matmuls slice exactly the 8 live rows).
        rq, lq = [], []
        for q in range(16):
            rt = qpool.tile([128, 8, 64], F32, tag=f"rq{q}")
            lt = qpool.tile([128, 8, 64], F32, tag=f"lq{q}")
            for bq in range(4):
                b = 4 * q + bq
                nc.sync.dma_start(
                    out=rt[32 * bq : 32 * bq + 8],
                    in_=R[:, b, 8 * co : 8 * co + 8, :],
                )
                nc.sync.dma_start(
                    out=lt[32 * bq : 32 * bq + 8],
                    in_=LT[:, b, 8 * co : 8 * co + 8, :],
                )
            rq.append(rt)
            lq.append(lt)

        for cpl in range(4):  # c-pair within octet
            cp = 4 * co + cpl  # absolute c-pair -> wt rows [128*cp, +128)
            slab = slabs.tile([128, 64, 64], F32, tag="slab")  # [(c2,d), a, b]
            for ot in range(8):  # b-octet: b in [8*ot, 8*ot+8)
                pb = apsum.tile([128, 512], F32, tag="apb")
                for cc in range(2):
                    ci = 2 * cpl + cc  # c index within octet
                    for bl in range(8):
                        q = 2 * ot + bl // 4
                        bq = bl % 4
                        nc.tensor.matmul(
                            pb[64 * cc : 64 * cc + 64, 64 * bl : 64 * bl + 64],
                            lhsT=rq[q][32 * bq : 32 * bq + 8, ci, :],
                            rhs=lq[q][32 * bq : 32 * bq + 8, ci, :],
                            start=True,
                            stop=True,
                            tile_position=(32 * bq, 64 * cc),
                        )
                # pb cols are (bl, a); scatter into slab cols (a, b=8*ot+bl)
                nc.any.tensor_copy(
                    out=slab[:, :, 8 * ot : 8 * ot + 8],
                    in_=pb[:, :].rearrange("p (bl a) -> p a bl", bl=8),
                )
            nc.sync.dma_start(out=wt[128 * cp : 128 * cp + 128, :], in_=slab[:, :, :])


def _emit_assembly_safe(ctx, tc, LT, R, wt, wdt):
    """Phase A variant with no tile_position packing: every matmul is a plain
    base-partition-0 [k=8, m=64, n=64] op; per-c-parity PSUM/slab halves so
    no cross-partition copies are needed."""
    nc = tc.nc
    lpool = ctx.enter_context(tc.tile_pool(name="low", bufs=1))
    slabs = ctx.enter_context(tc.tile_pool(name="slab", bufs=2))
    apsum = ctx.enter_context(tc.tile_pool(name="apsum", bufs=4, space="PSUM"))

    for cp in range(32):  # c-pair: c in {2cp, 2cp+1}; wt rows [128*cp, +128)
        rlow = lpool.tile([8, 64, 2, 64], F32, tag="rlow", name=f"rl{cp}")
        llow = lpool.tile([8, 64, 2, 64], F32, tag="llow", name=f"ll{cp}")
        nc.sync.dma_start(out=rlow, in_=R[:, :, 2 * cp : 2 * cp + 2, :])
        nc.sync.dma_start(out=llow, in_=LT[:, :, 2 * cp : 2 * cp + 2, :])
        sl = []
        for cc in range(2):
            s = slabs.tile([64, 64, 64], wdt, tag=f"slab{cc}", name=f"sl{cp}_{cc}")
            sl.append(s)
        for ot in range(8):  # b-octet
            pbs = []
            for cc in range(2):
                pb = apsum.tile(
                    [64, 512], F32, tag=f"apb{cc}", name=f"pb{cp}_{ot}_{cc}"
                )
                pbs.append(pb)
            for cc in range(2):
                for bl in range(8):
                    b = 8 * ot + bl
                    nc.tensor.matmul(
                        pbs[cc][:, 64 * bl : 64 * bl + 64],
                        lhsT=rlow[:, b, cc, :],
                        rhs=llow[:, b, cc, :],
                        start=True,
                        stop=True,
                    )
            for cc in range(2):
                nc.any.tensor_copy(
                    out=sl[cc][:, :, 8 * ot : 8 * ot + 8],
                    in_=pbs[cc][:, :].rearrange("p (bl a) -> p a bl", bl=8),
                )
        for cc in range(2):
            nc.sync.dma_start(
                out=wt[128 * cp + 64 * cc : 128 * cp + 64 * cc + 64, :],
                in_=sl[cc][:, :, :],
            )


def _emit_main(ctx, tc, x, bias_sb, ident, wt, out, wdt):
    """Phase B: out = X @ wt + bias (X transposed on-chip via PE)."""
    nc = tc.nc

    xt_pool = ctx.enter_context(tc.tile_pool(name="xt", bufs=1))
    xin_pool = ctx.enter_context(tc.tile_pool(name="xin", bufs=2))
    wq_pool = ctx.enter_context(tc.tile_pool(name="wq", bufs=2))
    out_pool = ctx.enter_context(tc.tile_pool(name="osb", bufs=3))
    psum = ctx.enter_context(tc.tile_pool(name="mpsum", bufs=8, space="PSUM"))

    n_chunks = S // S_CHUNK
    n_sb = S_CHUNK // 128  # s-blocks per chunk

    for ch in range(n_chunks):
        s0 = ch * S_CHUNK

        # ---- B1: build Xt[i, s] for this chunk (PE transpose) ----
        xt = xt_pool.tile([128, 32, S_CHUNK], wdt, tag="xt")  # [i_p, kt, s]
        for sb in range(n_sb):
            row = s0 + 128 * sb
            xin = []
            for h in range(2):
                xh = xin_pool.tile([128, 2048], F32, tag="xin", name=f"xin{ch}_{sb}_{h}")
                nc.sync.dma_start(
                    out=xh, in_=x[row : row + 128, 2048 * h : 2048 * h + 2048]
                )
                xin.append(xh)
            for kg in range(8):  # groups of 4 i-tiles
                tp = psum.tile([128, 512], F32, tag="pb")
                for j in range(4):
                    kt = 4 * kg + j
                    h, col = divmod(kt, 16)
                    nc.tensor.transpose(
                        tp[:, 128 * j : 128 * j + 128],
                        in_=xin[h][:, 128 * col : 128 * col + 128],
                        identity=ident,
                    )
                nc.any.tensor_copy(
                    out=xt[:, 4 * kg : 4 * kg + 4, 128 * sb : 128 * sb + 128],
                    in_=tp[:, :].rearrange("p (j s) -> p j s", j=4),
                )

        # ---- B2: tiled matmul against wt streamed from DRAM ----
        for ob in range(8):  # o-block of 512 cols
            ps = [
                psum.tile([128, 512], F32, tag="pb", name=f"ps{ob}_{sb}")
                for sb in range(n_sb)
            ]
            for kg in range(8):  # k-groups of 512 (4 k-tiles)
                wq = wq_pool.tile([128, 4, 512], wdt, tag="wq")
                nc.sync.dma_start(
                    out=wq,
                    in_=wt[
                        512 * kg : 512 * kg + 512, 512 * ob : 512 * ob + 512
                    ].rearrange("(kt p) o -> p kt o", p=128),
                )
                for j in range(4):
                    kt = 4 * kg + j
                    first = kt == 0
                    last = kt == 31
                    for sb in range(n_sb):
                        nc.tensor.matmul(
                            ps[sb],
                            lhsT=xt[:, kt, 128 * sb : 128 * sb + 128],
                            rhs=wq[:, j, :],
                            start=first,
                            stop=last,
                        )
            for sb in range(n_sb):
                ot = out_pool.tile([128, 512], F32, tag="osb")
                nc.any.tensor_add(
                    out=ot, in0=ps[sb], in1=bias_sb[:, 512 * ob : 512 * ob + 512]
                )
                nc.sync.dma_start(
                    out=out[
                        s0 + 128 * sb : s0 + 128 * sb + 128,
                        512 * ob : 512 * ob + 512,
                    ],
                    in_=ot,
                )


def build_nc():
    nc = bacc.Bacc(trn_type="TRN2", target_bir_lowering=False)
    x = nc.dram_tensor("x", [S, KD], F32, kind="ExternalInput")
    LT = nc.dram_tensor("LT", [8, 64, 64, 64], F32, kind="ExternalInput")
    R = nc.dram_tensor("R", [8, 64, 64, 64], F32, kind="ExternalInput")
    bias = nc.dram_tensor("bias", [OD], F32, kind="ExternalInput")
    out = nc.dram_tensor("out", [S, OD], F32, kind="ExternalOutput")
    wdt = _mm_dt()
    wt = nc.dram_tensor("wt", [KD, OD], wdt, kind="Internal")

    with tile.TileContext(nc) as tc:
        with ExitStack() as ctx:
            consts = ctx.enter_context(tc.tile_pool(name="consts", bufs=1))
            ident = consts.tile([128, 128], F32, tag="ident")
            make_identity(nc, ident)
            bias_sb = consts.tile([128, OD], F32, tag="bias")
            bap = bias[:]
            nc.gpsimd.dma_start(
                out=bias_sb,
                in_=bass.AP(tensor=bap.tensor, offset=bap.offset,
                            ap=[[0, 128]] + list(bap.ap)),
            )

            if not os.environ.get("BTT2_SKIP_A"):
                asm = (
                    _emit_assembly_safe
                    if os.environ.get("BTT2_ASM") == "safe"
                    else _emit_assembly
                )
                with ExitStack() as actx:
                    asm(actx, tc, LT, R, wt, wdt)
                tc.strict_bb_all_engine_barrier()
            if not os.environ.get("BTT2_SKIP_B"):
                with ExitStack() as bctx:
                    _emit_main(bctx, tc, x, bias_sb, ident, wt, out, wdt)
            else:
                # touch `out` so the ExternalOutput stays alive
                with tc.tile_pool(name="stub", bufs=1) as sp:
                    st = sp.tile([128, 512], wdt, tag="st")
                    nc.sync.dma_start(out=st, in_=wt[0:128, 0:512])
                    nc.sync.dma_start(
                        out=out[0:128, 0:512], in_=st[:, :].bitcast(F32)
                    )
    nc.finalize()
    return nc


_NC = None


def _get_nc():
    global _NC
    if _NC is None:
        _NC = build_nc()
    return _NC


def _run(x, L, R, bias, **spmd_kwargs):
    x = np.ascontiguousarray(x, dtype=np.float32)
    LT = np.ascontiguousarray(np.transpose(np.asarray(L, dtype=np.float32), (3, 1, 2, 0)))
    R = np.ascontiguousarray(R, dtype=np.float32)
    bias = np.ascontiguousarray(bias, dtype=np.float32)
    in_maps = [
        {"x": x[c], "LT": LT, "R": R, "bias": bias} for c in range(N_CORES)
    ]
    res = run_bass_kernel_spmd(
        _get_nc(), in_maps, core_ids=list(range(N_CORES)), **spmd_kwargs
    )
    outs = np.stack([res.results[c]["out"] for c in range(N_CORES)], axis=0)
    return outs, res


def kernel(x, L, R, bias):
    outs, _ = _run(x, L, R, bias)
    return outs
